# revision 1
# baseline (speedup 1.0000x reference)
"""Bridgeout FC layer (dense_mlp) Trainium2 kernel.

out[b, o] = sum_i x[b,i] * (w[i,o] + |w[i,o]| * noise[b,i,o]) + bias[o]

Strategy (8 NeuronCores, batch-parallel):
  - Each core owns 8 of the 64 samples: its x rows and noise slices.
  - Per core the dominant traffic is its 32 MB noise slice; everything is
    structured so DMA of noise is the roofline (~358 GB/s/core).
  - Layout: contraction index i on partitions, in 8 chunks of 128. One
    (128, 8*1024) f32 tile holds a full sample's noise; one DVE
    tensor_tensor multiplies it by |w| in place; the contraction with
    x[b, :] runs on the tensor engine as float32r matmuls (1 cycle/row)
    that accumulate x@w, bias and the noise term into a single
    (8, 512)-per-half PSUM group.
"""

import os
from contextlib import ExitStack

import numpy as np

import concourse.bass as bass
import concourse.mybir as mybir
import concourse.tile as tile
from concourse.bass_utils import run_bass_kernel_spmd

F32 = mybir.dt.float32
F32R = mybir.dt.float32r
ABS = mybir.ActivationFunctionType.Abs
COPY = mybir.ActivationFunctionType.Copy

N_CORES = 8
BS, IN_F, OUT_F = 64, 1024, 1024
BL = BS // N_CORES  # samples per core
P = 128  # SBUF partitions
NCH = IN_F // P  # contraction chunks of 128
HF = 512  # psum half width (one fp32 bank)
NHALF = OUT_F // HF

# Matmul dtype: float32r streams at 1 col/cycle (vs 4 for float32) at N>=256.
MM_DT = F32R

CG = 4  # contraction chunks per noise DMA / product tile (2 MB transfers)
NG = 2  # groups per sample (CG * NG == NCH)
NOISE_BUFS = 6
PROD_BUFS = 2


def _split_multi_waits(nc: bass.Bass) -> None:
    """walrus codegen on this toolchain accepts at most ONE sync-wait per
    instruction. Tile emits joins with several waits; hoist all but the last
    onto standalone EventSemaphore instructions (what wait_ge lowers to)
    immediately before the instruction, on the same engine stream."""
    n = 0
    for func in nc.m.functions:
        for block in func.blocks:
            out = []
            changed = False
            for inst in block.instructions:
                si = inst.sync_info
                if si is not None and si.on_wait and len(si.on_wait) > 1:
                    waits = list(si.on_wait)
                    for k, w in enumerate(waits[:-1]):
                        ev = mybir.InstEventSemaphore(
                            name=f"{inst.name}-sw{k}",
                            engine=inst.engine,
                            sync_info=mybir.SyncInfo(on_wait=[w], on_update=[]),
                        )
                        nc.register_instruction(ev)
                        out.append(ev)
                        n += 1
                    inst.sync_info = mybir.SyncInfo(
                        on_wait=[waits[-1]], on_update=list(si.on_update or [])
                    )
                    changed = True
                out.append(inst)
            if changed:
                block.instructions = out


def build_bass() -> bass.Bass:
    nc = bass.Bass(trn_type="TRN2", target_bir_lowering=False, debug=False)

    x_d = nc.dram_tensor("x", [BL, IN_F], F32, kind="ExternalInput").ap()
    w_d = nc.dram_tensor("weight", [IN_F, OUT_F], F32, kind="ExternalInput").ap()
    b_d = nc.dram_tensor("bias", [OUT_F], F32, kind="ExternalInput").ap()
    n_d = nc.dram_tensor("noise", [BL, IN_F, OUT_F], F32, kind="ExternalInput").ap()
    o_d = nc.dram_tensor("out", [BL, OUT_F], F32, kind="ExternalOutput").ap()

    with tile.TileContext(nc) as tc, ExitStack() as ctx:
        const = ctx.enter_context(tc.tile_pool(name="const", bufs=1))
        psump = ctx.enter_context(tc.tile_pool(name="psum", bufs=3, space="PSUM"))

        noisep = ctx.enter_context(tc.tile_pool(name="noise", bufs=NOISE_BUFS))
        prodp = ctx.enter_context(tc.tile_pool(name="prod", bufs=PROD_BUFS))
        outp = ctx.enter_context(tc.tile_pool(name="outp", bufs=2))

        # Contraction index mapping: i = p*NCH + c, so each partition's
        # slice of a chunk group is a long contiguous DRAM run (16-32 KB
        # descriptors instead of 4 KB) for both weight and noise DMAs.
        # Layout [p, (c o)] with i = p*NCH + c. The weight loads in two
        # halves, staged through the noise pool's buffers (they recycle for
        # noise immediately after |w| and the f32r copy are produced).
        wq_sb = const.tile([P, NCH * OUT_F], F32)
        w_r = const.tile([P, NCH * OUT_F], MM_DT)
        w_src = w_d.rearrange("(p c) o -> p c o", c=NCH)
        GW = NCH // NG
        for g in range(NG):
            lo, hi = g * GW * OUT_F, (g + 1) * GW * OUT_F
            wt = noisep.tile([P, GW * OUT_F], F32, name="nt", tag="nt")
            nc.sync.dma_start(
                wt[:].rearrange("p (c o) -> p c o", c=GW),
                w_src[:, g * GW : (g + 1) * GW, :],
            )
            nc.scalar.activation(wq_sb[:, lo:hi], wt[:], ABS)
            nc.vector.tensor_copy(w_r[:, lo:hi], wt[:])

        # x transposed: xT[p, j*NCH + c] = x[j, p*NCH + c]. Emitted after
        # the weight DMAs: its many tiny descriptors must not delay them.
        xT = const.tile([P, BL * NCH], F32)
        nc.sync.dma_start(
            xT[:].rearrange("p (j c) -> p j c", c=NCH),
            x_d.rearrange("j (p c) -> p j c", c=NCH),
        )
        xT_r = const.tile([P, BL * NCH], MM_DT)
        nc.vector.tensor_copy(xT_r[:], xT[:])

        bias_r = const.tile([1, OUT_F], MM_DT)
        nc.gpsimd.dma_start(bias_r[:], b_d.rearrange("(u o) -> u o", u=1))
        ones_f = const.tile([1, 1], F32)
        nc.vector.memset(ones_f[:], 1.0)
        ones = const.tile([1, 1], MM_DT)
        nc.vector.tensor_copy(ones[:], ones_f[:])

        # One sample at a time, in NG groups of CG contraction chunks; per
        # sample, per 512-wide half, one PSUM accumulation group holds
        # x@w + bias + the noise term.
        for j in range(BL):
            accs = [
                psump.tile([1, HF], F32, name=f"acc{j}_{h}", tag=f"acc{h}")
                for h in range(NHALF)
            ]
            # Last sample runs at half granularity to shorten the drain tail.
            cg = CG if j < BL - 1 else CG // 2
            for g in range(NCH // cg):
                nt = noisep.tile([P, cg * OUT_F], F32, name="nt", tag="nt")
                nc.sync.dma_start(
                    nt[:].rearrange("p (c o) -> p c o", c=cg),
                    n_d[j].rearrange("(p c) o -> p c o", c=NCH)[
                        :, g * cg : (g + 1) * cg, :
                    ],
                )
                pt = prodp.tile([P, cg * OUT_F], MM_DT, name="pt", tag="pt")
                nc.vector.tensor_mul(
                    pt[:],
                    nt[:],
                    wq_sb[:, g * cg * OUT_F : (g + 1) * cg * OUT_F],
                )
                for h in range(NHALF):
                    for cl in range(cg):
                        c = g * cg + cl
                        lhsT = xT_r[:, j * NCH + c : j * NCH + c + 1]
                        nc.tensor.matmul(
                            accs[h][:, :],
                            lhsT=lhsT,
                            rhs=w_r[
                                :, c * OUT_F + h * HF : c * OUT_F + h * HF + HF
                            ],
                            start=(c == 0),
                            stop=False,
                        )
                        nc.tensor.matmul(
                            accs[h][:, :],
                            lhsT=lhsT,
                            rhs=pt[
                                :, cl * OUT_F + h * HF : cl * OUT_F + h * HF + HF
                            ],
                            start=False,
                            stop=False,
                        )
            out_sb = outp.tile([1, OUT_F], F32, name=f"out{j}", tag="out")
            for h in range(NHALF):
                # bias via K=1 matmul closes the group
                nc.tensor.matmul(
                    accs[h][:, :],
                    lhsT=ones[:],
                    rhs=bias_r[:, h * HF : (h + 1) * HF],
                    start=False,
                    stop=True,
                )
                nc.scalar.activation(
                    out_sb[:, h * HF : (h + 1) * HF], accs[h][:, :], COPY
                )
            nc.sync.dma_start(o_d[j : j + 1, :], out_sb[:])

    _split_multi_waits(nc)
    return nc


def make_in_maps(x, weight, bias, noise):
    x = np.ascontiguousarray(x, dtype=np.float32)
    weight = np.ascontiguousarray(weight, dtype=np.float32)
    bias = np.ascontiguousarray(bias, dtype=np.float32)
    noise = np.ascontiguousarray(noise, dtype=np.float32)
    return [
        {
            "x": x[k * BL : (k + 1) * BL],
            "weight": weight,
            "bias": bias,
            "noise": np.ascontiguousarray(noise[k * BL : (k + 1) * BL]),
        }
        for k in range(N_CORES)
    ]


def kernel(**inputs) -> np.ndarray:
    nc = build_bass()
    in_maps = make_in_maps(
        inputs["x"], inputs["weight"], inputs["bias"], inputs["noise"]
    )
    res = run_bass_kernel_spmd(nc, in_maps, core_ids=list(range(N_CORES)))
    return np.concatenate(
        [res.results[k]["out"] for k in range(N_CORES)], axis=0
    ).astype(np.float32)


if __name__ == "__main__":
    rng = np.random.default_rng(0)
    x = rng.standard_normal((BS, IN_F), dtype=np.float32)
    w = rng.standard_normal((IN_F, OUT_F), dtype=np.float32) * 0.03
    b = rng.standard_normal((OUT_F,), dtype=np.float32) * 0.03
    s = (rng.random((BS, IN_F, OUT_F)) < 0.5).astype(np.float32) * 2 - 1
    out = kernel(x=x, weight=w, bias=b, noise=s)
    ref = np.einsum("bi,bio->bo", x, w[None] + np.abs(w)[None] * s) + b
    err = np.abs(out - ref).max() / np.abs(ref).max()
    print("rel err:", err)



# revision 5
# speedup vs baseline: 1.7612x; 1.7612x over previous
"""Bridgeout FC layer (dense_mlp) Trainium2 kernel.

out[b, o] = sum_i x[b,i] * (w[i,o] + |w[i,o]| * noise[b,i,o]) + bias[o]

Strategy (8 NeuronCores, contraction-parallel):
  - Each core owns a 128-row slice of the contraction index i. It reads
    noise[:, islice, :] (its 32 MB share of the 256 MB noise tensor),
    weight[islice, :] (0.5 MB -- NOT replicated as batch sharding would),
    and x[:, islice], and produces partial sums
    partial[b, o] = sum_{i in islice} x[b,i]*(w+|w|*noise)[b,i,o].
    The host adds the 8 partials plus the bias.
  - The noise slice is pre-cast to float16 and pre-transposed to
    [i, b, o] layout on the host. The 2e-2 rel-err gate leaves ~40x
    margin over fp16's 0.05% per-element error, and halving the bytes
    halves the DMA roofline: 16 MB/core at ~360 GB/s ~= 46 us. The
    [i, b, o] layout makes each SBUF partition line a G*2KB contiguous
    DRAM run, so DMA descriptors stay large.
  - Per group of G samples: one f16 DVE multiply pt = |w| (*) noise
    (16-bit packed operands run at 2x = ~246 G elem/s), then per
    (sample, 128-wide o-chunk) ONE matmul with the product as the
    STATIONARY operand (lhsT [128i, 128o]) and the x column as the
    moving operand: the result is a [128o, 1] psum COLUMN, so all 64
    samples pack into one [128, 64] psum tile per o-chunk (matmul psum
    writes must start at partition 0/32/64 -- row-per-sample layouts
    can't pack). The x@w term seeds each psum tile with one batched
    f32r matmul (lhsT = w chunk, rhs = all of xT). Epilogue: 8 wide
    scalar-engine copies psum -> SBUF, output written as outT [o, b].
"""

import numpy as np

from contextlib import ExitStack

import concourse.bass as bass
import concourse.mybir as mybir
import concourse.tile as tile
from concourse.bass_utils import run_bass_kernel_spmd

F32 = mybir.dt.float32
F32R = mybir.dt.float32r
F16 = mybir.dt.float16
ABS = mybir.ActivationFunctionType.Abs
COPY = mybir.ActivationFunctionType.Copy

N_CORES = 8
BS, IN_F, OUT_F = 64, 1024, 1024
P = 128  # SBUF partitions; also the per-core contraction slice
OC = OUT_F // P  # 128-wide output chunks

G = 8  # samples per noise DMA / product tile (2 MB f16 transfers)
NG = BS // G
NOISE_BUFS = 4
PROD_BUFS = 2


def _split_multi_waits(nc: bass.Bass) -> None:
    """walrus codegen on this toolchain accepts at most ONE sync-wait per
    instruction. Tile emits joins with several waits; hoist all but the last
    onto standalone EventSemaphore instructions (what wait_ge lowers to)
    immediately before the instruction, on the same engine stream."""
    for func in nc.m.functions:
        for block in func.blocks:
            out = []
            changed = False
            for inst in block.instructions:
                si = inst.sync_info
                if si is not None and si.on_wait and len(si.on_wait) > 1:
                    waits = list(si.on_wait)
                    for k, w in enumerate(waits[:-1]):
                        ev = mybir.InstEventSemaphore(
                            name=f"{inst.name}-sw{k}",
                            engine=inst.engine,
                            sync_info=mybir.SyncInfo(on_wait=[w], on_update=[]),
                        )
                        nc.register_instruction(ev)
                        out.append(ev)
                    inst.sync_info = mybir.SyncInfo(
                        on_wait=[waits[-1]], on_update=list(si.on_update or [])
                    )
                    changed = True
                out.append(inst)
            if changed:
                block.instructions = out


def build_bass() -> bass.Bass:
    nc = bass.Bass(trn_type="TRN2", target_bir_lowering=False, debug=False)

    xT_d = nc.dram_tensor("xt", [P, BS], F32, kind="ExternalInput").ap()
    w_d = nc.dram_tensor("weight", [P, OUT_F], F32, kind="ExternalInput").ap()
    n_d = nc.dram_tensor("noise", [P, BS, OUT_F], F16, kind="ExternalInput").ap()
    # outT[o, b]: psum partition dim is o, so the natural output layout is
    # transposed; the host transposes back.
    o_d = nc.dram_tensor("outT", [OUT_F, BS], F32, kind="ExternalOutput").ap()

    with tile.TileContext(nc) as tc, ExitStack() as ctx:
        const = ctx.enter_context(tc.tile_pool(name="const", bufs=1))
        psump = ctx.enter_context(tc.tile_pool(name="psum", bufs=1, space="PSUM"))
        noisep = ctx.enter_context(tc.tile_pool(name="noise", bufs=NOISE_BUFS))
        prodp = ctx.enter_context(tc.tile_pool(name="prod", bufs=PROD_BUFS))
        outp = ctx.enter_context(tc.tile_pool(name="outp", bufs=2))

        # Weight slice: partition p <- w[p, :], 4 KB contiguous lines.
        w_sb = const.tile([P, OUT_F], F32)
        nc.gpsimd.dma_start(w_sb[:], w_d)
        wq_h = const.tile([P, OUT_F], F16)
        nc.scalar.activation(wq_h[:], w_sb[:], ABS)
        w_r = const.tile([P, OUT_F], F32R)
        nc.vector.tensor_copy(w_r[:], w_sb[:])

        # x slice, pre-transposed on host to [i, b].
        xT_sb = const.tile([P, BS], F32)
        nc.gpsimd.dma_start(xT_sb[:], xT_d)
        xT_h = const.tile([P, BS], F16)
        nc.vector.tensor_copy(xT_h[:], xT_sb[:])
        xT_r = const.tile([P, BS], F32R)
        nc.vector.tensor_copy(xT_r[:], xT_sb[:])

        # One [128, 64] f32 psum accumulator per o-chunk; seeded with
        # xw.T[oc] = w[:, oc].T @ xT (all 64 samples in one matmul).
        accs = []
        for oc in range(OC):
            ps = psump.tile([P, BS], F32, name=f"ps{oc}", tag=f"ps{oc}")
            nc.tensor.matmul(
                ps[:, :],
                lhsT=w_r[:, oc * P : (oc + 1) * P],
                rhs=xT_r[:, :],
                start=True,
                stop=False,
                skip_group_check=True,
            )
            accs.append(ps)

        for g in range(NG):
            s0 = g * G
            nt = noisep.tile([P, G * OUT_F], F16, name="nt", tag="nt")
            nc.sync.dma_start(
                nt[:].rearrange("p (j o) -> p j o", j=G),
                n_d[:, s0 : s0 + G, :],
            )
            pt = prodp.tile([P, G * OUT_F], F16, name="pt", tag="pt")
            nc.vector.tensor_tensor(
                pt[:].rearrange("p (j o) -> p j o", j=G),
                nt[:].rearrange("p (j o) -> p j o", j=G),
                wq_h[:]
                .rearrange("p (u o) -> p u o", u=1)
                .broadcast_to((P, G, OUT_F)),
                mybir.AluOpType.mult,
            )
            # Per sample: 8 matmuls, product chunk stationary, x column
            # moving; each writes one psum column.
            for j in range(G):
                b = s0 + j
                last = g == NG - 1 and j == G - 1
                for oc in range(OC):
                    nc.tensor.matmul(
                        accs[oc][:, b : b + 1],
                        lhsT=pt[:, j * OUT_F + oc * P : j * OUT_F + (oc + 1) * P],
                        rhs=xT_h[:, b : b + 1],
                        start=False,
                        stop=last,
                        skip_group_check=True,
                    )

        for oc in range(OC):
            out_sb = outp.tile([P, BS], F32, name=f"osb{oc}", tag="osb")
            nc.scalar.activation(out_sb[:, :], accs[oc][:, :], COPY)
            nc.gpsimd.dma_start(o_d[oc * P : (oc + 1) * P, :], out_sb[:])

    _split_multi_waits(nc)
    return nc


def make_in_maps(x, weight, bias, noise):
    x = np.ascontiguousarray(x, dtype=np.float32)
    weight = np.ascontiguousarray(weight, dtype=np.float32)
    in_maps = []
    for k in range(N_CORES):
        sl = slice(k * P, (k + 1) * P)
        in_maps.append(
            {
                "xt": np.ascontiguousarray(x[:, sl].T),
                "weight": np.ascontiguousarray(weight[sl, :]),
                "noise": np.ascontiguousarray(
                    noise[:, sl, :].transpose(1, 0, 2).astype(np.float16)
                ),
            }
        )
    return in_maps


def assemble(results, bias) -> np.ndarray:
    acc = np.zeros((BS, OUT_F), dtype=np.float64)
    for k in range(N_CORES):
        acc += results[k]["outT"].T.astype(np.float64)
    acc += np.asarray(bias, dtype=np.float64)[None, :]
    return acc.astype(np.float32)


def kernel(**inputs) -> np.ndarray:
    nc = build_bass()
    in_maps = make_in_maps(
        inputs["x"], inputs["weight"], inputs["bias"], inputs["noise"]
    )
    res = run_bass_kernel_spmd(nc, in_maps, core_ids=list(range(N_CORES)))
    return assemble(res.results, inputs["bias"])


if __name__ == "__main__":
    rng = np.random.default_rng(0)
    x = rng.standard_normal((BS, IN_F), dtype=np.float32)
    w = rng.standard_normal((IN_F, OUT_F), dtype=np.float32) * 0.03
    b = rng.standard_normal((OUT_F,), dtype=np.float32) * 0.03
    s = (rng.random((BS, IN_F, OUT_F)) < 0.5).astype(np.float32) * 2 - 1
    out = kernel(x=x, weight=w, bias=b, noise=s)
    ref = np.einsum("bi,bio->bo", x, w[None] + np.abs(w)[None] * s) + b
    err = np.abs(out - ref).max() / np.abs(ref).max()
    print("rel err:", err)


# revision 6
# speedup vs baseline: 1.7925x; 1.0178x over previous
"""Bridgeout FC layer (dense_mlp) Trainium2 kernel.

out[b, o] = sum_i x[b,i] * (w[i,o] + |w[i,o]| * noise[b,i,o]) + bias[o]

Strategy (8 NeuronCores, contraction-parallel):
  - Each core owns a 128-row slice of the contraction index i. It reads
    noise[:, islice, :] (its 32 MB share of the 256 MB noise tensor) and
    weight[islice, :] (0.5 MB -- NOT replicated as batch sharding would)
    and produces partial[b, o] = sum_{i in islice} x*(w+|w|*noise); the
    host adds the 8 partials plus the bias.
  - noise ships as float16 (the 2e-2 rel-err gate leaves ~40x margin
    over fp16's 0.05% element error), halving DMA bytes: 16 MB/core at
    the measured ~420 GB/s aggregate DMA rate ~= 40 us.
  - Block-diagonal matmuls: the i-slice is split into 8 sub-slices of
    16; the host interleaves noise so SBUF partition j*16+u holds
    sample (g*8+j)'s sub-row u. Then lhsT[128, 8] is a block-diagonal
    x matrix (zero blocks kill cross-sample terms) and ONE matmul
    computes a [8 samples, 512] psum block while streaming 512 f16
    columns/cycle-class -- wide on both M and N. Naive alternatives
    lose: per-sample moving-product matmuls (M=1, N=512) leave 64
    narrow [1,512] psum tiles whose copies serialize ~55 us on the
    scalar engine (matmul psum writes must start at partition 0/32/64,
    so rows can't pack); per-sample stationary-product matmuls (M=128,
    N=1) pay a ~170 ns pipeline+LDWEIGHTS cost per single streamed
    column (measured 79 us kernel, PE-bound).
  - The product pt = |w| (*) noise runs on the DVE in f16 (16-bit
    packed operands run at 2x): ~34 us total, under the DMA time.
    |w| ships pre-replicated in the interleaved layout (2 MB f16).
  - The x@w term seeds each group's psum block with one batched f32r
    matmul (lhsT = xT columns) before the noise matmuls accumulate.
  - Groups of 8 samples process in half-group (4-sample-row) DMA/DVE
    chunks for a finer software pipeline and shorter drain tail.
"""

import numpy as np

from contextlib import ExitStack

import concourse.bass as bass
import concourse.mybir as mybir
import concourse.tile as tile
from concourse.bass_utils import run_bass_kernel_spmd

F32 = mybir.dt.float32
F32R = mybir.dt.float32r
F16 = mybir.dt.float16
COPY = mybir.ActivationFunctionType.Copy

N_CORES = 8
BS, IN_F, OUT_F = 64, 1024, 1024
P = 128  # SBUF partitions; also the per-core contraction slice
HF = 512  # one fp32 psum bank
M = 8  # samples per matmul / group
SUB = P // M  # contraction sub-slice per sample within a matmul
NG = BS // M  # groups
NT = M  # t-tiles per group (one per contraction sub-slice)
GF = M * OUT_F  # free size of one group's noise tile
HB = GF // 2  # half-group free size (DMA/DVE chunk)
NOISE_BUFS = 4
PROD_BUFS = 3


def _split_multi_waits(nc: bass.Bass) -> None:
    """walrus codegen on this toolchain accepts at most ONE sync-wait per
    instruction. Tile emits joins with several waits; hoist all but the last
    onto standalone EventSemaphore instructions (what wait_ge lowers to)
    immediately before the instruction, on the same engine stream."""
    for func in nc.m.functions:
        for block in func.blocks:
            out = []
            changed = False
            for inst in block.instructions:
                si = inst.sync_info
                if si is not None and si.on_wait and len(si.on_wait) > 1:
                    waits = list(si.on_wait)
                    for k, w in enumerate(waits[:-1]):
                        ev = mybir.InstEventSemaphore(
                            name=f"{inst.name}-sw{k}",
                            engine=inst.engine,
                            sync_info=mybir.SyncInfo(on_wait=[w], on_update=[]),
                        )
                        nc.register_instruction(ev)
                        out.append(ev)
                    inst.sync_info = mybir.SyncInfo(
                        on_wait=[waits[-1]], on_update=list(si.on_update or [])
                    )
                    changed = True
                out.append(inst)
            if changed:
                block.instructions = out


def build_bass() -> bass.Bass:
    nc = bass.Bass(trn_type="TRN2", target_bir_lowering=False, debug=False)

    xT_d = nc.dram_tensor("xt", [P, BS], F32, kind="ExternalInput").ap()
    w_d = nc.dram_tensor("weight", [P, OUT_F], F32, kind="ExternalInput").ap()
    wq_d = nc.dram_tensor("wqrep", [P, GF], F16, kind="ExternalInput").ap()
    xb_d = nc.dram_tensor("xblk", [P, NG * NT * M], F16, kind="ExternalInput").ap()
    n_d = nc.dram_tensor("noise", [NG, P, GF], F16, kind="ExternalInput").ap()
    o_d = nc.dram_tensor("out", [BS, OUT_F], F32, kind="ExternalOutput").ap()

    with tile.TileContext(nc) as tc, ExitStack() as ctx:
        const = ctx.enter_context(tc.tile_pool(name="const", bufs=1))
        psump = ctx.enter_context(tc.tile_pool(name="psum", bufs=2, space="PSUM"))
        noisep = ctx.enter_context(tc.tile_pool(name="noise", bufs=NOISE_BUFS))
        prodp = ctx.enter_context(tc.tile_pool(name="prod", bufs=PROD_BUFS))
        outp = ctx.enter_context(tc.tile_pool(name="outp", bufs=2))

        # |w| pre-replicated into the interleaved (j,u) x (t,o) layout,
        # halves on the gpsimd queue so the first DVE mult unblocks early.
        wq_sb = const.tile([P, GF], F16)
        nc.gpsimd.dma_start(wq_sb[:, :HB], wq_d[:, :HB])
        nc.gpsimd.dma_start(wq_sb[:, HB:], wq_d[:, HB:])

        # Small constants on the scalar queue, off the critical DMA paths.
        w_sb = const.tile([P, OUT_F], F32)
        nc.scalar.dma_start(w_sb[:], w_d)
        xT_sb = const.tile([P, BS], F32)
        nc.scalar.dma_start(xT_sb[:], xT_d)
        xblk = const.tile([P, NG * NT * M], F16)
        nc.scalar.dma_start(xblk[:], xb_d)

        w_r = const.tile([P, OUT_F], F32R)
        nc.vector.tensor_copy(w_r[:], w_sb[:])
        xT_r = const.tile([P, BS], F32R)
        nc.vector.tensor_copy(xT_r[:], xT_sb[:])

        for g in range(NG):
            s0 = g * M
            # x@w seeds this group's psum blocks (batched f32r matmul).
            pss = []
            for h in range(2):
                ps = psump.tile([M, HF], F32, name=f"ps{g}_{h}", tag=f"ps{h}")
                nc.tensor.matmul(
                    ps[:, :],
                    lhsT=xT_r[:, s0 : s0 + M],
                    rhs=w_r[:, h * HF : (h + 1) * HF],
                    start=True,
                    stop=False,
                    skip_group_check=True,
                )
                pss.append(ps)
            for h2 in range(2):
                nt = noisep.tile([P, HB], F16, name="nt", tag="nt")
                nc.sync.dma_start(nt[:], n_d[g][:, h2 * HB : (h2 + 1) * HB])
                pt = prodp.tile([P, HB], F16, name="pt", tag="pt")
                nc.vector.tensor_tensor(
                    pt[:],
                    nt[:],
                    wq_sb[:, h2 * HB : (h2 + 1) * HB],
                    mybir.AluOpType.mult,
                )
                for tl in range(NT // 2):
                    t = h2 * (NT // 2) + tl
                    for h in range(2):
                        nc.tensor.matmul(
                            pss[h][:, :],
                            lhsT=xblk[:, (g * NT + t) * M : (g * NT + t + 1) * M],
                            rhs=pt[:, tl * OUT_F + h * HF : tl * OUT_F + h * HF + HF],
                            start=False,
                            stop=(t == NT - 1),
                            skip_group_check=True,
                        )
            out_sb = outp.tile([M, OUT_F], F32, name="osb", tag="osb")
            for h in range(2):
                nc.scalar.activation(
                    out_sb[:, h * HF : (h + 1) * HF], pss[h][:, :], COPY
                )
            nc.gpsimd.dma_start(o_d[s0 : s0 + M, :], out_sb[:])

    _split_multi_waits(nc)
    return nc


def make_in_maps(x, weight, bias, noise):
    x = np.ascontiguousarray(x, dtype=np.float32)
    weight = np.ascontiguousarray(weight, dtype=np.float32)
    in_maps = []
    for k in range(N_CORES):
        sl = slice(k * P, (k + 1) * P)
        w_k = weight[sl, :]  # [P, OUT_F]
        x_k = x[:, sl]  # [BS, P]

        # noise interleave: partition j*SUB+u <- sample g*M+j, i-row t*SUB+u,
        # free dim ordered (t, o).
        nv = np.ascontiguousarray(noise[:, sl, :], dtype=np.float32)
        nv = nv.reshape(NG, M, NT, SUB, OUT_F)  # [g, j, t, u, o]
        nv = nv.transpose(0, 1, 3, 2, 4).astype(np.float16)  # [g, j, u, t, o]
        nv = np.ascontiguousarray(nv).reshape(NG, P, GF)

        # |w| replicated over j in the same layout.
        wq = np.abs(w_k).reshape(NT, SUB, OUT_F).transpose(1, 0, 2)  # [u, t, o]
        wq = np.broadcast_to(wq[None], (M, SUB, NT, OUT_F)).astype(np.float16)
        wq = np.ascontiguousarray(wq).reshape(P, GF)

        # Block-diagonal x: xblk[j*SUB+u, ((g*NT+t)*M)+m] =
        #   x[g*M+m, t*SUB+u] if j == m else 0.
        xb = np.zeros((M, SUB, NG, NT, M), dtype=np.float16)
        xr = x_k.reshape(NG, M, NT, SUB)  # [g, j, t, u]
        for j in range(M):
            xb[j, :, :, :, j] = xr[:, j].transpose(2, 0, 1)  # [u, g, t]
        xb = xb.reshape(P, NG * NT * M)

        in_maps.append(
            {
                "xt": np.ascontiguousarray(x_k.T),
                "weight": np.ascontiguousarray(w_k),
                "wqrep": wq,
                "xblk": np.ascontiguousarray(xb),
                "noise": nv,
            }
        )
    return in_maps


def assemble(results, bias) -> np.ndarray:
    acc = np.zeros((BS, OUT_F), dtype=np.float64)
    for k in range(N_CORES):
        acc += results[k]["out"].astype(np.float64)
    acc += np.asarray(bias, dtype=np.float64)[None, :]
    return acc.astype(np.float32)


def kernel(**inputs) -> np.ndarray:
    nc = build_bass()
    in_maps = make_in_maps(
        inputs["x"], inputs["weight"], inputs["bias"], inputs["noise"]
    )
    res = run_bass_kernel_spmd(nc, in_maps, core_ids=list(range(N_CORES)))
    return assemble(res.results, inputs["bias"])


if __name__ == "__main__":
    rng = np.random.default_rng(0)
    x = rng.standard_normal((BS, IN_F), dtype=np.float32)
    w = rng.standard_normal((IN_F, OUT_F), dtype=np.float32) * 0.03
    b = rng.standard_normal((OUT_F,), dtype=np.float32) * 0.03
    s = (rng.random((BS, IN_F, OUT_F)) < 0.5).astype(np.float32) * 2 - 1
    out = kernel(x=x, weight=w, bias=b, noise=s)
    ref = np.einsum("bi,bio->bo", x, w[None] + np.abs(w)[None] * s) + b
    err = np.abs(out - ref).max() / np.abs(ref).max()
    print("rel err:", err)


# revision 8
# speedup vs baseline: 2.0364x; 1.1361x over previous
"""Bridgeout FC layer (dense_mlp) Trainium2 kernel.

out[b, o] = sum_i x[b,i] * (w[i,o] + |w[i,o]| * noise[b,i,o]) + bias[o]

Strategy (8 NeuronCores, contraction-parallel):
  - Each core owns a 128-row slice of the contraction index i. It reads
    noise[:, islice, :] (its 32 MB share of the 256 MB noise tensor) and
    weight[islice, :] (0.5 MB -- NOT replicated as batch sharding would)
    and produces partial[b, o] = sum_{i in islice} x*(w+|w|*noise); the
    host adds the 8 partials plus the bias.
  - noise ships as float16 (the 2e-2 rel-err gate leaves ~40x margin
    over fp16's 0.05% element error), halving DMA bytes: 16 MB/core at
    the measured ~420 GB/s aggregate DMA rate ~= 40 us.
  - Block-diagonal matmuls: the i-slice is split into 8 sub-slices of
    16; the host interleaves noise so SBUF partition j*16+u holds
    sample (g*8+j)'s sub-row u. Then lhsT[128, 8] is a block-diagonal
    x matrix (zero blocks kill cross-sample terms) and ONE matmul
    computes a [8 samples, 512] psum block while streaming 512 f16
    columns/cycle-class -- wide on both M and N. Naive alternatives
    lose: per-sample moving-product matmuls (M=1, N=512) leave 64
    narrow [1,512] psum tiles whose copies serialize ~55 us on the
    scalar engine (matmul psum writes must start at partition 0/32/64,
    so rows can't pack); per-sample stationary-product matmuls (M=128,
    N=1) pay a ~170 ns pipeline+LDWEIGHTS cost per single streamed
    column (measured 79 us kernel, PE-bound).
  - The product pt = |w| (*) noise runs on the DVE in f16 (16-bit
    packed operands run at 2x): ~34 us total, under the DMA time.
    |w| ships pre-replicated in the interleaved layout (2 MB f16).
  - The x@w term seeds each group's psum block with one batched f32r
    matmul (lhsT = xT columns) before the noise matmuls accumulate.
  - Groups of 8 samples process in half-group (4-sample-row) DMA/DVE
    chunks for a finer software pipeline and shorter drain tail.
"""

import numpy as np

from contextlib import ExitStack

import concourse.bass as bass
import concourse.mybir as mybir
import concourse.tile as tile
from concourse.bass_utils import run_bass_kernel_spmd

F32 = mybir.dt.float32
F32R = mybir.dt.float32r
F16 = mybir.dt.float16
COPY = mybir.ActivationFunctionType.Copy

N_CORES = 8
BS, IN_F, OUT_F = 64, 1024, 1024
P = 128  # SBUF partitions; also the per-core contraction slice
HF = 512  # one fp32 psum bank
M = 8  # samples per matmul / group
SUB = P // M  # contraction sub-slice per sample within a matmul
NG = BS // M  # groups
NT = M  # t-tiles per group (one per contraction sub-slice)
GF = M * OUT_F  # free size of one group's noise tile
HB = GF // 2  # half-group free size (DMA/DVE chunk)
NOISE_BUFS = 6
PROD_BUFS = 3


def _split_multi_waits(nc: bass.Bass) -> None:
    """walrus codegen on this toolchain accepts at most ONE sync-wait per
    instruction. Tile emits joins with several waits; hoist all but the last
    onto standalone EventSemaphore instructions (what wait_ge lowers to)
    immediately before the instruction, on the same engine stream."""
    for func in nc.m.functions:
        for block in func.blocks:
            out = []
            changed = False
            for inst in block.instructions:
                si = inst.sync_info
                if si is not None and si.on_wait and len(si.on_wait) > 1:
                    waits = list(si.on_wait)
                    for k, w in enumerate(waits[:-1]):
                        ev = mybir.InstEventSemaphore(
                            name=f"{inst.name}-sw{k}",
                            engine=inst.engine,
                            sync_info=mybir.SyncInfo(on_wait=[w], on_update=[]),
                        )
                        nc.register_instruction(ev)
                        out.append(ev)
                    inst.sync_info = mybir.SyncInfo(
                        on_wait=[waits[-1]], on_update=list(si.on_update or [])
                    )
                    changed = True
                out.append(inst)
            if changed:
                block.instructions = out


def build_bass() -> bass.Bass:
    nc = bass.Bass(trn_type="TRN2", target_bir_lowering=False, debug=False)

    xT_d = nc.dram_tensor("xt", [P, BS], F32, kind="ExternalInput").ap()
    w_d = nc.dram_tensor("weight", [P, OUT_F], F32, kind="ExternalInput").ap()
    wq_d = nc.dram_tensor("wqrep", [P, GF], F16, kind="ExternalInput").ap()
    xb_d = nc.dram_tensor("xblk", [P, NG * NT * M], F16, kind="ExternalInput").ap()
    n_d = nc.dram_tensor("noise", [NG, P, GF], F16, kind="ExternalInput").ap()
    o_d = nc.dram_tensor("out", [BS, OUT_F], F32, kind="ExternalOutput").ap()

    with tile.TileContext(nc) as tc, ExitStack() as ctx:
        const = ctx.enter_context(tc.tile_pool(name="const", bufs=1))
        psump = ctx.enter_context(tc.tile_pool(name="psum", bufs=2, space="PSUM"))
        noisep = ctx.enter_context(tc.tile_pool(name="noise", bufs=NOISE_BUFS))
        prodp = ctx.enter_context(tc.tile_pool(name="prod", bufs=PROD_BUFS))
        outp = ctx.enter_context(tc.tile_pool(name="outp", bufs=2))

        # Small constants go FIRST on the sync queue: behind the noise
        # stream on a side queue they starve for 10-20 us and the DVE
        # casts that depend on them stall the whole pipeline.
        w_sb = const.tile([P, OUT_F], F32)
        nc.sync.dma_start(w_sb[:], w_d)
        xT_sb = const.tile([P, BS], F32)
        nc.sync.dma_start(xT_sb[:], xT_d)
        xblk = const.tile([P, NG * NT * M], F16)
        nc.sync.dma_start(xblk[:], xb_d)

        # |w| pre-replicated into the interleaved (j,u) x (t,o) layout,
        # halves on the gpsimd queue so the first DVE mult unblocks early.
        wq_sb = const.tile([P, GF], F16)
        nc.gpsimd.dma_start(wq_sb[:, :HB], wq_d[:, :HB])
        nc.gpsimd.dma_start(wq_sb[:, HB:], wq_d[:, HB:])

        # f32r casts on the otherwise-idle Pool engine: the DVE must not
        # have anything in program order ahead of the first product mult.
        w_r = const.tile([P, OUT_F], F32R)
        nc.gpsimd.tensor_copy(w_r[:], w_sb[:])
        xT_r = const.tile([P, BS], F32R)
        nc.gpsimd.tensor_copy(xT_r[:], xT_sb[:])

        for g in range(NG):
            s0 = g * M
            # x@w seeds this group's psum blocks (batched f32r matmul).
            pss = []
            for h in range(2):
                ps = psump.tile([M, HF], F32, name=f"ps{g}_{h}", tag=f"ps{h}")
                nc.tensor.matmul(
                    ps[:, :],
                    lhsT=xT_r[:, s0 : s0 + M],
                    rhs=w_r[:, h * HF : (h + 1) * HF],
                    start=True,
                    stop=False,
                    skip_group_check=True,
                )
                pss.append(ps)
            # Last group runs at quarter granularity to shorten the
            # DMA->DVE->PE drain tail.
            nch = 2 if g < NG - 1 else 4
            tpc = NT // nch
            cf = GF // nch
            for c in range(nch):
                nt = noisep.tile([P, cf], F16, name="nt", tag="nt")
                nc.sync.dma_start(nt[:], n_d[g][:, c * cf : (c + 1) * cf])
                pt = prodp.tile([P, cf], F16, name="pt", tag="pt")
                nc.vector.tensor_tensor(
                    pt[:],
                    nt[:],
                    wq_sb[:, c * cf : (c + 1) * cf],
                    mybir.AluOpType.mult,
                )
                for tl in range(tpc):
                    t = c * tpc + tl
                    for h in range(2):
                        nc.tensor.matmul(
                            pss[h][:, :],
                            lhsT=xblk[:, (g * NT + t) * M : (g * NT + t + 1) * M],
                            rhs=pt[:, tl * OUT_F + h * HF : tl * OUT_F + h * HF + HF],
                            start=False,
                            stop=(t == NT - 1),
                            skip_group_check=True,
                        )
            out_sb = outp.tile([M, OUT_F], F32, name="osb", tag="osb")
            for h in range(2):
                nc.scalar.activation(
                    out_sb[:, h * HF : (h + 1) * HF], pss[h][:, :], COPY
                )
            nc.gpsimd.dma_start(o_d[s0 : s0 + M, :], out_sb[:])

    _split_multi_waits(nc)
    return nc


def make_in_maps(x, weight, bias, noise):
    x = np.ascontiguousarray(x, dtype=np.float32)
    weight = np.ascontiguousarray(weight, dtype=np.float32)
    in_maps = []
    for k in range(N_CORES):
        sl = slice(k * P, (k + 1) * P)
        w_k = weight[sl, :]  # [P, OUT_F]
        x_k = x[:, sl]  # [BS, P]

        # noise interleave: partition j*SUB+u <- sample g*M+j, i-row t*SUB+u,
        # free dim ordered (t, o).
        nv = np.ascontiguousarray(noise[:, sl, :], dtype=np.float32)
        nv = nv.reshape(NG, M, NT, SUB, OUT_F)  # [g, j, t, u, o]
        nv = nv.transpose(0, 1, 3, 2, 4).astype(np.float16)  # [g, j, u, t, o]
        nv = np.ascontiguousarray(nv).reshape(NG, P, GF)

        # |w| replicated over j in the same layout.
        wq = np.abs(w_k).reshape(NT, SUB, OUT_F).transpose(1, 0, 2)  # [u, t, o]
        wq = np.broadcast_to(wq[None], (M, SUB, NT, OUT_F)).astype(np.float16)
        wq = np.ascontiguousarray(wq).reshape(P, GF)

        # Block-diagonal x: xblk[j*SUB+u, ((g*NT+t)*M)+m] =
        #   x[g*M+m, t*SUB+u] if j == m else 0.
        xb = np.zeros((M, SUB, NG, NT, M), dtype=np.float16)
        xr = x_k.reshape(NG, M, NT, SUB)  # [g, j, t, u]
        for j in range(M):
            xb[j, :, :, :, j] = xr[:, j].transpose(2, 0, 1)  # [u, g, t]
        xb = xb.reshape(P, NG * NT * M)

        in_maps.append(
            {
                "xt": np.ascontiguousarray(x_k.T),
                "weight": np.ascontiguousarray(w_k),
                "wqrep": wq,
                "xblk": np.ascontiguousarray(xb),
                "noise": nv,
            }
        )
    return in_maps


def assemble(results, bias) -> np.ndarray:
    acc = np.zeros((BS, OUT_F), dtype=np.float64)
    for k in range(N_CORES):
        acc += results[k]["out"].astype(np.float64)
    acc += np.asarray(bias, dtype=np.float64)[None, :]
    return acc.astype(np.float32)


def kernel(**inputs) -> np.ndarray:
    nc = build_bass()
    in_maps = make_in_maps(
        inputs["x"], inputs["weight"], inputs["bias"], inputs["noise"]
    )
    res = run_bass_kernel_spmd(nc, in_maps, core_ids=list(range(N_CORES)))
    return assemble(res.results, inputs["bias"])


if __name__ == "__main__":
    rng = np.random.default_rng(0)
    x = rng.standard_normal((BS, IN_F), dtype=np.float32)
    w = rng.standard_normal((IN_F, OUT_F), dtype=np.float32) * 0.03
    b = rng.standard_normal((OUT_F,), dtype=np.float32) * 0.03
    s = (rng.random((BS, IN_F, OUT_F)) < 0.5).astype(np.float32) * 2 - 1
    out = kernel(x=x, weight=w, bias=b, noise=s)
    ref = np.einsum("bi,bio->bo", x, w[None] + np.abs(w)[None] * s) + b
    err = np.abs(out - ref).max() / np.abs(ref).max()
    print("rel err:", err)
